# revision 1
# baseline (speedup 1.0000x reference)
"""Bass/Trainium2 kernel for nn_CSEM sparse_attention problem.

Sharding: 8 cores = 4 samples x 2 spatial halves (bottom half vertically
flipped on host so all cores run an identical NEFF). Channel attention
needs two tiny pairwise AllReduces (l2 norms, then the 192x192 logits).
"""

import numpy as np
import ml_dtypes

import concourse.bass as bass
import concourse.mybir as mybir
import concourse.tile as tile
from concourse.bass_utils import run_bass_kernel_spmd
from concourse.masks import make_identity

BF16 = mybir.dt.bfloat16
F32 = mybir.dt.float32
BN_EPS = 1e-5
L2_EPS = 1e-12

CIN, C, C3 = 64, 192, 576
W, WP = 96, 98
XR = 54          # x slab rows (1 zero pad + 53 data)
YR = 52          # y rows computed locally (0..51)
TR = 50          # t rows 0..49
QR = 49          # conv1 out rows 0..48
SR = 48          # rows feeding S partial
OR_ = 48         # final output rows per core
GROUPS = [[0, 1], [2, 3], [4, 5], [6, 7]]

ENGINE_NOP_OPCODE = 159


def _split_waits(nc, limit=1):
    """This walrus build rejects instructions carrying more than one sem-wait
    command. Spread extra waits onto same-engine ENGINE_NOPs inserted just
    before the offending instruction (semantically identical: the engine
    blocks on each wait in program order)."""
    ctr = [0]
    for f in nc.m.functions:
        for blk in f.blocks:
            il = blk.instructions
            new = []
            for inst in il:
                si = inst.sync_info
                waits = list(si.on_wait) if (si and si.on_wait) else []
                if len(waits) > limit:
                    for w in waits[:-limit]:
                        ctr[0] += 1
                        nop = mybir.InstNoOp(name=f"WNOP-{ctr[0]}")
                        nop.engine = inst.engine
                        nop.sync_info = mybir.SyncInfo(on_wait=[w], on_update=[])
                        new.append(nop)
                    si.on_wait = waits[-limit:]
                new.append(inst)
            il[:] = new


def _row_blocks(nrows, per=5):
    out, r = [], 0
    while r < nrows:
        n = min(per, nrows - r)
        out.append((r, n))
        r += n
    return out


def build_nc():
    nc = bass.Bass()

    xs_d = nc.declare_dram_parameter("xs", [64, XR * WP], BF16, isOutput=False)
    w0_d = nc.declare_dram_parameter("w0t", [64, 9 * C], BF16, isOutput=False)
    sb0_d = nc.declare_dram_parameter("sb0", [C, 2], F32, isOutput=False)
    w1h_d = nc.declare_dram_parameter("w1th", [128, 9 * C3], BF16, isOutput=False)
    w1l_d = nc.declare_dram_parameter("w1tl", [128, 9 * C3], BF16, isOutput=False)
    sb1_d = nc.declare_dram_parameter("sb1", [C3, 2], F32, isOutput=False)
    w2a_d = nc.declare_dram_parameter("w2da", [128, 9 * 128], BF16, isOutput=False)
    w2b_d = nc.declare_dram_parameter("w2db", [128, 9 * 128], BF16, isOutput=False)
    b2_d = nc.declare_dram_parameter("b2v", [C, 1], F32, isOutput=False)
    tmp_d = nc.declare_dram_parameter("tempv", [1, 1], F32, isOutput=False)
    yout = nc.declare_dram_parameter("yout", [C, OR_ * W], F32, isOutput=True)

    cc1i = nc.dram_tensor("cc1i", [2, C], F32)
    cc1o = nc.dram_tensor("cc1o", [2, C], F32)
    cc2i = nc.dram_tensor("cc2i", [C, C], F32)
    cc2o = nc.dram_tensor("cc2o", [C, C], F32)

    with tile.TileContext(nc) as tc:
        _body(nc, tc, xs_d, w0_d, sb0_d, w1h_d, w1l_d, sb1_d, w2a_d, w2b_d,
              b2_d, tmp_d, yout, cc1i, cc1o, cc2i, cc2o)
    _split_waits(nc)
    return nc


def _body(nc, tc, xs_d, w0_d, sb0_d, w1h_d, w1l_d, sb1_d, w2a_d, w2b_d,
          b2_d, tmp_d, yout, cc1i, cc1o, cc2i, cc2o):
    import contextlib
    ctx = contextlib.ExitStack()
    P = ctx.enter_context(tc.tile_pool(name="persist", bufs=1))
    ev = ctx.enter_context(tc.tile_pool(name="evac", bufs=3))

    # ---- persistent SBUF ----
    xsb = P.tile([64, XR * WP], BF16, tag="xsb")
    w0sb = P.tile([64, 9 * C], BF16, tag="w0sb")
    w1hi = P.tile([128, 9 * C3], BF16, tag="w1hi")
    w1lo = P.tile([128, 9 * C3], BF16, tag="w1lo")
    w2da = P.tile([128, 9 * 128], BF16, tag="w2da")
    w2db = P.tile([128, 9 * 128], BF16, tag="w2db")
    sc0a = P.tile([128, 2], F32, tag="sc0a")
    sc0b = P.tile([64, 2], F32, tag="sc0b")
    scp = [128, 64, 128, 64, 128, 64]
    sc1 = [P.tile([scp[i], 2], F32, tag=f"sc1_{i}", name=f"sc1_{i}")
           for i in range(6)]
    b2a = P.tile([128, 1], F32, tag="b2a")
    b2b = P.tile([64, 1], F32, tag="b2b")
    tmps = P.tile([128, 1], F32, tag="tmps")

    nc.gpsimd.dma_start(out=xsb[:], in_=xs_d[:])
    nc.gpsimd.dma_start(out=w0sb[:], in_=w0_d[:])
    nc.gpsimd.dma_start(out=w1hi[:], in_=w1h_d[:])
    nc.gpsimd.dma_start(out=w1lo[:], in_=w1l_d[:])
    nc.gpsimd.dma_start(out=w2da[:], in_=w2a_d[:])
    nc.gpsimd.dma_start(out=w2db[:], in_=w2b_d[:])
    nc.gpsimd.dma_start(out=sc0a[:], in_=sb0_d[0:128, :])
    nc.gpsimd.dma_start(out=sc0b[:], in_=sb0_d[128:192, :])
    for i, (lo, hi) in enumerate([(0, 128), (128, 192), (192, 320),
                                  (320, 384), (384, 512), (512, 576)]):
        nc.gpsimd.dma_start(out=sc1[i][:], in_=sb1_d[lo:hi, :])
    nc.gpsimd.dma_start(out=b2a[:], in_=b2_d[0:128, :])
    nc.gpsimd.dma_start(out=b2b[:], in_=b2_d[128:192, :])
    nc.gpsimd.dma_start(
        out=tmps[:],
        in_=bass.AP(tensor=tmp_d, offset=0, ap=[[0, 128], [1, 1]]))

    ident = P.tile([128, 128], BF16, tag="ident")
    make_identity(nc, ident[:])

    xpool = P.tile([128, YR, WP], BF16, tag="xpool")
    ta = P.tile([128, TR + 1, WP], BF16, tag="ta")
    tb = P.tile([128, TR + 1, WP], BF16, tag="tb")
    brs = P.tile([128, TR + 1, WP], BF16, tag="brs")
    oa = P.tile([128, QR + 1, WP], BF16, tag="oa")
    ob = P.tile([128, QR + 1, WP], BF16, tag="ob")
    for buf in (ta, tb, brs, oa, ob):
        nc.vector.memset(buf[:], 0.0)

    qa = P.tile([128, SR * W], BF16, tag="qa")
    qb = P.tile([64, SR * W], BF16, tag="qb")
    ka = P.tile([128, SR * W], BF16, tag="ka")
    kb = P.tile([64, SR * W], BF16, tag="kb")
    va = P.tile([128, QR * W], BF16, tag="va")
    vb = P.tile([128, QR * W], BF16, tag="vb")
    nc.vector.memset(vb[64:128, :], 0.0)

    xsv = xsb.rearrange("p (r w) -> p r w", w=WP)
    w0v = w0sb.rearrange("p (t m) -> p t m", t=9)
    w1hv = w1hi.rearrange("p (t m) -> p t m", t=9)
    w1lv = w1lo.rearrange("p (t m) -> p t m", t=9)
    w2av = w2da.rearrange("p (t m) -> p t m", t=9)
    w2bv = w2db.rearrange("p (t m) -> p t m", t=9)

    # ---------------- conv0 ----------------
    with tc.tile_pool(name="ps_c0", bufs=4, space="PSUM") as pp0:
        for (r0, nr) in _row_blocks(YR):
            for mi, (m0, m1) in enumerate([(0, 128), (128, 192)]):
                ps = pp0.tile([128, 5, W], F32, tag="c0ps")
                mw = m1 - m0
                pb = 0 if mi == 0 else 64  # psum base partition (col tiling)
                for t in range(9):
                    dy, dx = t // 3 - 1, t % 3 - 1
                    nc.tensor.matmul(
                        ps[pb:pb + mw, 0:nr, :],
                        lhsT=w0v[:, t, m0:m1],
                        rhs=xsv[:, r0 + 1 + dy:r0 + 1 + dy + nr, 1 + dx:97 + dx],
                        start=(t == 0), stop=(t == 8))
                if mi == 0:
                    nc.scalar.activation(
                        out=xpool[0:64, r0:r0 + nr, 1:97], in_=ps[0:64, 0:nr, :],
                        func=mybir.ActivationFunctionType.Relu,
                        bias=sc0a[0:64, 1:2], scale=sc0a[0:64, 0:1])
                    if r0 < TR:
                        nc.scalar.activation(
                            out=ta[64:128, r0 + 1:r0 + 1 + nr, 1:97],
                            in_=ps[64:128, 0:nr, :],
                            func=mybir.ActivationFunctionType.Relu,
                            bias=sc0a[64:128, 1:2], scale=sc0a[64:128, 0:1])
                else:
                    nc.scalar.activation(
                        out=xpool[64:128, r0:r0 + nr, 1:97],
                        in_=ps[64:128, 0:nr, :],
                        func=mybir.ActivationFunctionType.Relu,
                        bias=sc0b[:, 1:2], scale=sc0b[:, 0:1])

    # ---------------- pools + bilinear ----------------
    plh = P.tile([128, YR, 48], BF16, tag="plh")
    pl = P.tile([128, 26, 48], BF16, tag="pl")
    vint = P.tile([128, TR, 48], BF16, tag="vint")
    nc.vector.tensor_tensor(out=plh[0:64], in0=xpool[0:64, :, 1:97:2],
                            in1=xpool[0:64, :, 2:98:2], op=mybir.AluOpType.max)
    nc.vector.tensor_tensor(out=plh[64:128], in0=xpool[64:128, :, 1:97:2],
                            in1=xpool[64:128, :, 2:98:2], op=mybir.AluOpType.add)
    nc.vector.tensor_tensor(out=pl[0:64], in0=plh[0:64, 0:52:2, :],
                            in1=plh[0:64, 1:52:2, :], op=mybir.AluOpType.max)
    nc.vector.tensor_tensor(out=pl[64:128], in0=plh[64:128, 0:52:2, :],
                            in1=plh[64:128, 1:52:2, :], op=mybir.AluOpType.add)

    # vertical bilinear (t rows 0..49)
    nc.vector.tensor_copy(out=vint[:, 0, :], in_=pl[:, 0, :])
    tmpv = P.tile([128, 25, 48], BF16, tag="tmpv")
    nc.vector.tensor_scalar(out=tmpv[:], in0=pl[:, 1:26, :], scalar1=0.25,
                            scalar2=None, op0=mybir.AluOpType.mult)
    nc.vector.scalar_tensor_tensor(
        out=vint[:, 1:50:2, :], in0=pl[:, 0:25, :], scalar=0.75,
        in1=tmpv[:], op0=mybir.AluOpType.mult, op1=mybir.AluOpType.add)
    nc.vector.tensor_scalar(out=tmpv[:, 0:24, :], in0=pl[:, 1:25, :], scalar1=0.75,
                            scalar2=None, op0=mybir.AluOpType.mult)
    nc.vector.scalar_tensor_tensor(
        out=vint[:, 2:49:2, :], in0=pl[:, 0:24, :], scalar=0.25,
        in1=tmpv[:, 0:24, :], op0=mybir.AluOpType.mult, op1=mybir.AluOpType.add)

    # horizontal bilinear into brs rows 1..50 cols 1..96 (br1 lower, br3 upper;
    # avgpool's 0.25 folded into the upper-partition constants)
    cA = P.tile([128, 2], F32, tag="cA")
    nc.vector.memset(cA[0:64, 0:1], 0.75)
    nc.vector.memset(cA[0:64, 1:2], 0.25)
    nc.vector.memset(cA[64:128, 0:1], 0.1875)
    nc.vector.memset(cA[64:128, 1:2], 0.0625)
    cC = P.tile([128, 1], F32, tag="cC")
    nc.vector.memset(cC[0:64, :], 1.0)
    nc.vector.memset(cC[64:128, :], 0.25)

    nc.vector.tensor_scalar(out=brs[:, 1:51, 1:2], in0=vint[:, :, 0:1],
                            scalar1=cC[:, 0:1], scalar2=None,
                            op0=mybir.AluOpType.mult)
    nc.vector.tensor_scalar(out=brs[:, 1:51, 96:97], in0=vint[:, :, 47:48],
                            scalar1=cC[:, 0:1], scalar2=None,
                            op0=mybir.AluOpType.mult)
    tmph = P.tile([128, TR, 47], BF16, tag="tmph")
    nc.vector.tensor_scalar(out=tmph[:], in0=vint[:, :, 1:48], scalar1=cA[:, 1:2],
                            scalar2=None, op0=mybir.AluOpType.mult)
    nc.vector.scalar_tensor_tensor(
        out=brs[:, 1:51, 2:96:2], in0=vint[:, :, 0:47], scalar=cA[:, 0:1],
        in1=tmph[:], op0=mybir.AluOpType.mult, op1=mybir.AluOpType.add)
    nc.vector.tensor_scalar(out=tmph[:], in0=vint[:, :, 1:48], scalar1=cA[:, 0:1],
                            scalar2=None, op0=mybir.AluOpType.mult)
    nc.vector.scalar_tensor_tensor(
        out=brs[:, 1:51, 3:96:2], in0=vint[:, :, 0:47], scalar=cA[:, 1:2],
        in1=tmph[:], op0=mybir.AluOpType.mult, op1=mybir.AluOpType.add)

    nc.gpsimd.dma_start(out=ta[0:64, 1:51, :], in_=brs[0:64, 1:51, :])
    nc.gpsimd.dma_start(out=tb[0:64, 1:51, :], in_=brs[64:128, 1:51, :])

    # ---------------- conv1 + attention prologue ----------------
    qk_blocks = _row_blocks(SR)
    v_blocks = _row_blocks(QR)
    grp_dst = [
        (0, 128, qa, sc1[0], qk_blocks),
        (128, 192, qb, sc1[1], qk_blocks),
        (192, 320, ka, sc1[2], qk_blocks),
        (320, 384, kb, sc1[3], qk_blocks),
        (384, 512, va, sc1[4], v_blocks),
        (512, 576, vb, sc1[5], v_blocks),
    ]

    with tc.tile_pool(name="ps_c1", bufs=2, space="PSUM") as pp1, \
         tc.tile_pool(name="ps_tr", bufs=2, space="PSUM") as ppt, \
         tc.tile_pool(name="ps_s", bufs=1, space="PSUM") as pps:

        def conv1_group(gi):
            m0, m1, dst, sc, blocks = grp_dst[gi]
            mw = m1 - m0
            dstv = dst.rearrange("p (r w) -> p r w", w=W)
            for (r0, nr) in blocks:
                ps = pp1.tile([128, 5, W], F32, tag="c1ps")
                first = True
                for t in range(9):
                    dy, dx = t // 3 - 1, t % 3 - 1
                    for (wv, rhs) in ((w1hv, ta), (w1lv, tb)):
                        nc.tensor.matmul(
                            ps[0:mw, 0:nr, :],
                            lhsT=wv[:, t, m0:m1],
                            rhs=rhs[:, r0 + 1 + dy:r0 + 1 + dy + nr,
                                    1 + dx:97 + dx],
                            start=first, stop=(t == 8 and rhs is tb))
                        first = False
                nc.scalar.activation(
                    out=dstv[0:mw, r0:r0 + nr, :], in_=ps[0:mw, 0:nr, :],
                    func=mybir.ActivationFunctionType.Relu,
                    bias=sc[:, 1:2], scale=sc[:, 0:1])

        for gi in range(4):
            conv1_group(gi)

        # sumsq(q,k) + AllReduce #1 (overlaps conv1 v groups)
        sq = P.tile([128, SR * W], BF16, tag="sq")
        qsqa = P.tile([128, 1], F32, tag="qsqa")
        qsqb = P.tile([64, 1], F32, tag="qsqb")
        ksqa = P.tile([128, 1], F32, tag="ksqa")
        ksqb = P.tile([64, 1], F32, tag="ksqb")
        for src, dst in ((qa, qsqa), (qb, qsqb), (ka, ksqa), (kb, ksqb)):
            p = src.shape[0]
            nc.vector.tensor_tensor(out=sq[0:p], in0=src[:], in1=src[:],
                                    op=mybir.AluOpType.mult)
            nc.vector.reduce_sum(out=dst[:], in_=sq[0:p],
                                 axis=mybir.AxisListType.X)
        nc.gpsimd.dma_start(out=cc1i[0, 0:128], in_=qsqa[:, 0])
        nc.gpsimd.dma_start(out=cc1i[0, 128:192], in_=qsqb[:, 0])
        nc.gpsimd.dma_start(out=cc1i[1, 0:128], in_=ksqa[:, 0])
        nc.gpsimd.dma_start(out=cc1i[1, 128:192], in_=ksqb[:, 0])
        nc.gpsimd.collective_compute(
            "AllReduce", mybir.AluOpType.add, replica_groups=GROUPS,
            ins=[cc1i[:]], outs=[cc1o[:]])

        for gi in range(4, 6):
            conv1_group(gi)

        # global norms -> scale k in place
        rsa = P.tile([128, 2], F32, tag="rsa")
        rsb = P.tile([64, 2], F32, tag="rsb")
        nc.gpsimd.dma_start(out=rsa[:], in_=bass.AP(
            tensor=cc1o, offset=0, ap=[[1, 128], [C, 2]]))
        nc.gpsimd.dma_start(out=rsb[:], in_=bass.AP(
            tensor=cc1o, offset=128, ap=[[1, 64], [C, 2]]))
        for rs in (rsa, rsb):
            nc.scalar.activation(out=rs[:], in_=rs[:],
                                 func=mybir.ActivationFunctionType.Sqrt)
            nc.vector.tensor_scalar(out=rs[:], in0=rs[:], scalar1=float(L2_EPS),
                                    scalar2=None, op0=mybir.AluOpType.max)
            nc.vector.reciprocal(out=rs[:], in_=rs[:])
        nc.vector.tensor_scalar(out=ka[:], in0=ka[:], scalar1=rsa[:, 1:2],
                                scalar2=None, op0=mybir.AluOpType.mult)
        nc.vector.tensor_scalar(out=kb[:], in0=kb[:], scalar1=rsb[:, 1:2],
                                scalar2=None, op0=mybir.AluOpType.mult)

        # transposes + S partial
        NCH = SR * W // 128
        spa = pps.tile([128, C], F32, tag="spa")
        spb = pps.tile([64, C], F32, tag="spb")
        qav = qa.rearrange("p (c k) -> p c k", k=128)
        qbv = qb.rearrange("p (c k) -> p c k", k=128)
        kav = ka.rearrange("p (c k) -> p c k", k=128)
        kbv = kb.rearrange("p (c k) -> p c k", k=128)
        for ci in range(NCH):
            tq = ppt.tile([128, C], BF16, tag="tq")
            tk = ppt.tile([128, C], BF16, tag="tk")
            nc.tensor.transpose(tq[:, 0:128], qav[:, ci, :], ident[:])
            nc.tensor.transpose(tq[:, 128:192], qbv[:, ci, :], ident[0:64, 0:64])
            nc.tensor.transpose(tk[:, 0:128], kav[:, ci, :], ident[:])
            nc.tensor.transpose(tk[:, 128:192], kbv[:, ci, :], ident[0:64, 0:64])
            qtc = ev.tile([128, C], BF16, tag="qtc")
            ktc = ev.tile([128, C], BF16, tag="ktc")
            nc.scalar.copy(out=qtc[:], in_=tq[:])
            nc.scalar.copy(out=ktc[:], in_=tk[:])
            nc.tensor.matmul(spa[:], lhsT=qtc[:, 0:128], rhs=ktc[:],
                             start=(ci == 0), stop=(ci == NCH - 1))
            nc.tensor.matmul(spb[:], lhsT=qtc[:, 128:192], rhs=ktc[:],
                             start=(ci == 0), stop=(ci == NCH - 1))
        ssa = P.tile([128, C], F32, tag="ssa")
        ssb = P.tile([64, C], F32, tag="ssb")
        nc.scalar.copy(out=ssa[:], in_=spa[:])
        nc.scalar.copy(out=ssb[:], in_=spb[:])
        nc.gpsimd.dma_start(out=cc2i[0:128, :], in_=ssa[:])
        nc.gpsimd.dma_start(out=cc2i[128:192, :], in_=ssb[:])
        nc.gpsimd.collective_compute(
            "AllReduce", mybir.AluOpType.add, replica_groups=GROUPS,
            ins=[cc2i[:]], outs=[cc2o[:]])

    # ---------------- softmax + P^T ----------------
    sfa = P.tile([128, C], F32, tag="sfa")
    sfb = P.tile([64, C], F32, tag="sfb")
    nc.gpsimd.dma_start(out=sfa[:], in_=cc2o[0:128, :])
    nc.gpsimd.dma_start(out=sfb[:], in_=cc2o[128:192, :])
    paf = P.tile([128, C], BF16, tag="paf")
    pbf = P.tile([64, C], BF16, tag="pbf")
    for sf, rs, pf in ((sfa, rsa, paf), (sfb, rsb, pbf)):
        p = sf.shape[0]
        rqt = ev.tile([128, 1], F32, tag="rqt")
        mx = ev.tile([128, 1], F32, tag="mx")
        sm = ev.tile([128, 1], F32, tag="sm")
        nc.vector.tensor_tensor(out=rqt[0:p], in0=rs[:, 0:1], in1=tmps[0:p],
                                op=mybir.AluOpType.mult)
        nc.vector.tensor_scalar(out=sf[:], in0=sf[:], scalar1=rqt[0:p],
                                scalar2=None, op0=mybir.AluOpType.mult)
        nc.vector.reduce_max(out=mx[0:p], in_=sf[:], axis=mybir.AxisListType.X)
        nc.vector.tensor_scalar(out=mx[0:p], in0=mx[0:p], scalar1=-1.0,
                                scalar2=None, op0=mybir.AluOpType.mult)
        nc.scalar.activation(out=sf[:], in_=sf[:],
                             func=mybir.ActivationFunctionType.Exp,
                             bias=mx[0:p], scale=1.0, accum_out=sm[0:p])
        nc.vector.reciprocal(out=sm[0:p], in_=sm[0:p])
        nc.vector.tensor_scalar(out=pf[:], in0=sf[:], scalar1=sm[0:p],
                                scalar2=None, op0=mybir.AluOpType.mult)

    pta = P.tile([128, C], BF16, tag="pta")
    ptb = P.tile([128, C], BF16, tag="ptb")
    nc.vector.memset(ptb[:], 0.0)
    with tc.tile_pool(name="ps_pt", bufs=2, space="PSUM") as ppm, \
         tc.tile_pool(name="ps_pv", bufs=2, space="PSUM") as ppv:
        tp1 = ppm.tile([128, C], BF16, tag="tp1")
        nc.tensor.transpose(tp1[:, 0:128], paf[:, 0:128], ident[:])
        nc.tensor.transpose(tp1[:, 128:192], pbf[:, 0:128], ident[0:64, 0:64])
        nc.scalar.copy(out=pta[:], in_=tp1[:])
        tp2 = ppm.tile([128, C], BF16, tag="tp1")
        nc.tensor.transpose(tp2[0:64, 0:128], paf[:, 128:192], ident[:])
        nc.tensor.transpose(tp2[0:64, 128:192], pbf[:, 128:192],
                            ident[0:64, 0:64])
        nc.scalar.copy(out=ptb[0:64, :], in_=tp2[0:64, :])

        # out = P @ v
        vav = va.rearrange("p (r w) -> p r w", w=W)
        vbv = vb.rearrange("p (r w) -> p r w", w=W)
        for (r0, nr) in v_blocks:
            po = ppv.tile([128, 5, W], F32, tag="po")
            po2 = ppv.tile([128, 5, W], F32, tag="po2")
            nc.tensor.matmul(po[:, 0:nr, :], lhsT=pta[:, 0:128],
                             rhs=vav[:, r0:r0 + nr, :], start=True, stop=False)
            nc.tensor.matmul(po[:, 0:nr, :], lhsT=ptb[:, 0:128],
                             rhs=vbv[:, r0:r0 + nr, :], start=False, stop=True)
            nc.tensor.matmul(po2[0:64, 0:nr, :], lhsT=pta[:, 128:192],
                             rhs=vav[:, r0:r0 + nr, :], start=True, stop=False)
            nc.tensor.matmul(po2[0:64, 0:nr, :], lhsT=ptb[:, 128:192],
                             rhs=vbv[:, r0:r0 + nr, :], start=False, stop=True)
            nc.scalar.copy(out=oa[:, r0 + 1:r0 + 1 + nr, 1:97],
                           in_=po[:, 0:nr, :])
            nc.scalar.copy(out=ob[0:64, r0 + 1:r0 + 1 + nr, 1:97],
                           in_=po2[0:64, 0:nr, :])

    # ---------------- depthwise conv + bias ----------------
    yv = yout.rearrange("c (r w) -> c r w", w=W)
    with tc.tile_pool(name="ps_dw", bufs=4, space="PSUM") as ppd:
        for (r0, nr) in _row_blocks(OR_):
            for (wv, src, b2t, mw, o0) in ((w2av, oa, b2a, 128, 0),
                                           (w2bv, ob, b2b, 64, 128)):
                ps = ppd.tile([128, 5, W], F32, tag="dwps")
                for t in range(9):
                    dy, dx = t // 3 - 1, t % 3 - 1
                    nc.tensor.matmul(
                        ps[0:mw, 0:nr, :],
                        lhsT=wv[:, t, 0:mw],
                        rhs=src[:, r0 + 1 + dy:r0 + 1 + dy + nr, 1 + dx:97 + dx],
                        start=(t == 0), stop=(t == 8))
                fo = ev.tile([128, 5, W], F32, tag="fo")
                nc.scalar.activation(out=fo[0:mw, 0:nr, :], in_=ps[0:mw, 0:nr, :],
                                     func=mybir.ActivationFunctionType.Identity,
                                     bias=b2t[:, 0:1], scale=1.0)
                nc.gpsimd.dma_start(out=yv[o0:o0 + mw, r0:r0 + nr, :],
                                    in_=fo[0:mw, 0:nr, :])
    ctx.close()


# ---------------- host side ----------------
_NC_CACHE = None


def _get_nc():
    global _NC_CACHE
    if _NC_CACHE is None:
        _NC_CACHE = build_nc()
    return _NC_CACHE


def _pack_weights(inp, flip):
    bf = ml_dtypes.bfloat16
    w0 = inp["w0"][:, :, ::-1, :] if flip else inp["w0"]
    w1 = inp["w1"][:, :, ::-1, :] if flip else inp["w1"]
    w2 = inp["w2"][:, :, ::-1, :] if flip else inp["w2"]

    w0t = np.zeros((64, 9, C), np.float32)
    for t in range(9):
        w0t[:, t] = w0[:, :, t // 3, t % 3].T
    s0 = inp["g0"] / np.sqrt(inp["v0"] + BN_EPS)
    t0 = inp["be0"] + (inp["b0"] - inp["m0"]) * s0
    sb0 = np.stack([s0, t0], axis=1).astype(np.float32)

    w1th = np.zeros((128, 9, C3), np.float32)
    w1tl = np.zeros((128, 9, C3), np.float32)
    for t in range(9):
        w1th[:, t] = w1[:, 0:128, t // 3, t % 3].T
        w1tl[0:64, t] = w1[:, 128:192, t // 3, t % 3].T
    s1 = inp["g1"] / np.sqrt(inp["v1"] + BN_EPS)
    t1 = inp["be1"] + (inp["b1"] - inp["m1"]) * s1
    sb1 = np.stack([s1, t1], axis=1).astype(np.float32)

    w2da = np.zeros((128, 9, 128), np.float32)
    w2db = np.zeros((128, 9, 128), np.float32)
    r64, r128 = np.arange(64), np.arange(128)
    for t in range(9):
        d = w2[:, 0, t // 3, t % 3]
        w2da[r128, t, r128] = d[0:128]
        w2db[r64, t, r64] = d[128:192]

    return {
        "w0t": np.ascontiguousarray(w0t.reshape(64, 9 * C)).astype(bf),
        "sb0": sb0,
        "w1th": np.ascontiguousarray(w1th.reshape(128, 9 * C3)).astype(bf),
        "w1tl": np.ascontiguousarray(w1tl.reshape(128, 9 * C3)).astype(bf),
        "sb1": sb1,
        "w2da": np.ascontiguousarray(w2da.reshape(128, 9 * 128)).astype(bf),
        "w2db": np.ascontiguousarray(w2db.reshape(128, 9 * 128)).astype(bf),
        "b2v": inp["b2"].reshape(C, 1).astype(np.float32),
    }


def kernel(**inputs):
    inputs = {k: np.asarray(v) for k, v in inputs.items()}
    x = inputs["x"]
    B = x.shape[0]
    bf = ml_dtypes.bfloat16
    packs = [_pack_weights(inputs, flip) for flip in (False, True)]
    tempv = np.asarray(inputs["temp"], np.float32).reshape(1, 1)

    in_maps = []
    for core in range(8):
        s, h = core // 2, core % 2
        xi = x[s]
        if h:
            xi = xi[:, ::-1, :]
        slab = np.zeros((64, XR, WP), np.float32)
        slab[:, 1:54, 1:97] = xi[:, 0:53, :]
        m = dict(packs[h])
        m["xs"] = np.ascontiguousarray(slab.reshape(64, XR * WP)).astype(bf)
        m["tempv"] = tempv
        in_maps.append(m)

    nc = _get_nc()
    res = run_bass_kernel_spmd(nc, in_maps, list(range(8)))
    out = np.zeros((B, C, 96, 96), np.float32)
    for core in range(8):
        s, h = core // 2, core % 2
        yc = res.results[core]["yout"].reshape(C, OR_, W)
        if h:
            out[s, :, 48:96] = yc[:, ::-1, :]
        else:
            out[s, :, 0:48] = yc
    return out



# revision 3
# speedup vs baseline: 5601.6858x; 5601.6858x over previous
"""Bass/Trainium2 kernel for nn_CSEM sparse_attention problem.

Sharding: 8 cores = 4 samples x 2 spatial halves (bottom half vertically
flipped on host so all cores run an identical NEFF). Channel attention
needs two tiny pairwise AllReduces (l2 norms, then the 192x192 logits).
"""

import numpy as np
import ml_dtypes

import concourse.bass as bass
import concourse.mybir as mybir
import concourse.tile as tile
from concourse.bass_utils import run_bass_kernel_spmd
from concourse.masks import make_identity

BF16 = mybir.dt.bfloat16
F32 = mybir.dt.float32
BN_EPS = 1e-5
L2_EPS = 1e-12

CIN, C, C3 = 64, 192, 576
W, WP = 96, 98
XR = 54          # x slab rows (1 zero pad + 53 data)
YR = 52          # y rows computed locally (0..51)
TR = 50          # t rows 0..49
QR = 49          # conv1 out rows 0..48
SR = 48          # rows feeding S partial
OR_ = 48         # final output rows per core
GROUPS = [[0, 1], [2, 3], [4, 5], [6, 7]]

ENGINE_NOP_OPCODE = 159


def _split_waits(nc, limit=1):
    """This walrus build rejects instructions carrying more than one sem-wait
    command. Spread extra waits onto same-engine ENGINE_NOPs inserted just
    before the offending instruction (semantically identical: the engine
    blocks on each wait in program order)."""
    ctr = [0]
    for f in nc.m.functions:
        for blk in f.blocks:
            il = blk.instructions
            new = []
            for inst in il:
                si = inst.sync_info
                waits = list(si.on_wait) if (si and si.on_wait) else []
                if len(waits) > limit:
                    for w in waits[:-limit]:
                        ctr[0] += 1
                        nop = mybir.InstNoOp(name=f"WNOP-{ctr[0]}")
                        nop.engine = inst.engine
                        nop.sync_info = mybir.SyncInfo(on_wait=[w], on_update=[])
                        new.append(nop)
                    si.on_wait = waits[-limit:]
                new.append(inst)
            il[:] = new


def _row_blocks(nrows, per=5):
    out, r = [], 0
    while r < nrows:
        n = min(per, nrows - r)
        out.append((r, n))
        r += n
    return out


def build_nc():
    nc = bass.Bass()

    xs_d = nc.declare_dram_parameter("xs", [64, XR * WP], BF16, isOutput=False)
    w0_d = nc.declare_dram_parameter("w0t", [64, 9 * C], BF16, isOutput=False)
    sb0_d = nc.declare_dram_parameter("sb0", [C, 2], F32, isOutput=False)
    w1h_d = nc.declare_dram_parameter("w1th", [128, 9 * C3], BF16, isOutput=False)
    w1l_d = nc.declare_dram_parameter("w1tl", [128, 9 * C3], BF16, isOutput=False)
    sb1_d = nc.declare_dram_parameter("sb1", [C3, 2], F32, isOutput=False)
    w2a_d = nc.declare_dram_parameter("w2da", [128, 9 * 128], BF16, isOutput=False)
    w2b_d = nc.declare_dram_parameter("w2db", [128, 9 * 128], BF16, isOutput=False)
    b2_d = nc.declare_dram_parameter("b2v", [C, 1], F32, isOutput=False)
    tmp_d = nc.declare_dram_parameter("tempv", [1, 1], F32, isOutput=False)
    yout = nc.declare_dram_parameter("yout", [C, OR_ * W], F32, isOutput=True)

    cc1i = nc.dram_tensor("cc1i", [2, C], F32)
    cc1o = nc.dram_tensor("cc1o", [2, C], F32)
    cc2i = nc.dram_tensor("cc2i", [C, C], F32)
    cc2o = nc.dram_tensor("cc2o", [C, C], F32)

    with tile.TileContext(nc) as tc:
        _body(nc, tc, xs_d, w0_d, sb0_d, w1h_d, w1l_d, sb1_d, w2a_d, w2b_d,
              b2_d, tmp_d, yout, cc1i, cc1o, cc2i, cc2o)
    _split_waits(nc)
    return nc


def _body(nc, tc, xs_d, w0_d, sb0_d, w1h_d, w1l_d, sb1_d, w2a_d, w2b_d,
          b2_d, tmp_d, yout, cc1i, cc1o, cc2i, cc2o):
    import contextlib
    ctx = contextlib.ExitStack()
    P = ctx.enter_context(tc.tile_pool(name="persist", bufs=1))
    ev = ctx.enter_context(tc.tile_pool(name="evac", bufs=3))

    # ---- persistent SBUF ----
    xsb = P.tile([64, XR * WP], BF16, tag="xsb")
    w0sb = P.tile([64, 9 * C], BF16, tag="w0sb")
    w1hi = P.tile([128, 9 * C3], BF16, tag="w1hi")
    w1lo = P.tile([128, 9 * C3], BF16, tag="w1lo")
    w2da = P.tile([128, 9 * 128], BF16, tag="w2da")
    w2db = P.tile([128, 9 * 128], BF16, tag="w2db")
    sc0a = P.tile([128, 2], F32, tag="sc0a")
    sc0b = P.tile([64, 2], F32, tag="sc0b")
    scp = [128, 64, 128, 64, 128, 64]
    sc1 = [P.tile([scp[i], 2], F32, tag=f"sc1_{i}", name=f"sc1_{i}")
           for i in range(6)]
    b2a = P.tile([128, 1], F32, tag="b2a")
    b2b = P.tile([64, 1], F32, tag="b2b")
    tmps = P.tile([128, 1], F32, tag="tmps")

    nc.gpsimd.dma_start(out=xsb[:], in_=xs_d[:])
    nc.gpsimd.dma_start(out=w0sb[:], in_=w0_d[:])
    nc.gpsimd.dma_start(out=w1hi[:], in_=w1h_d[:])
    nc.gpsimd.dma_start(out=w1lo[:], in_=w1l_d[:])
    nc.gpsimd.dma_start(out=w2da[:], in_=w2a_d[:])
    nc.gpsimd.dma_start(out=w2db[:], in_=w2b_d[:])
    nc.gpsimd.dma_start(out=sc0a[:], in_=sb0_d[0:128, :])
    nc.gpsimd.dma_start(out=sc0b[:], in_=sb0_d[128:192, :])
    for i, (lo, hi) in enumerate([(0, 128), (128, 192), (192, 320),
                                  (320, 384), (384, 512), (512, 576)]):
        nc.gpsimd.dma_start(out=sc1[i][:], in_=sb1_d[lo:hi, :])
    nc.gpsimd.dma_start(out=b2a[:], in_=b2_d[0:128, :])
    nc.gpsimd.dma_start(out=b2b[:], in_=b2_d[128:192, :])
    nc.gpsimd.dma_start(
        out=tmps[:],
        in_=bass.AP(tensor=tmp_d, offset=0, ap=[[0, 128], [1, 1]]))

    ident = P.tile([128, 128], BF16, tag="ident")
    make_identity(nc, ident[:])

    xpool = P.tile([128, YR, WP], BF16, tag="xpool")
    ta = P.tile([128, TR + 1, WP], BF16, tag="ta")
    tb = P.tile([128, TR + 1, WP], BF16, tag="tb")
    brs = P.tile([128, TR + 1, WP], BF16, tag="brs")
    oa = P.tile([128, QR + 1, WP], BF16, tag="oa")
    ob = P.tile([128, QR + 1, WP], BF16, tag="ob")
    for buf in (ta, tb, brs, oa, ob):
        nc.vector.memset(buf[:], 0.0)

    qa = P.tile([128, SR * W], BF16, tag="qa")
    qb = P.tile([64, SR * W], BF16, tag="qb")
    ka = P.tile([128, SR * W], BF16, tag="ka")
    kb = P.tile([64, SR * W], BF16, tag="kb")
    va = P.tile([128, QR * W], BF16, tag="va")
    vb = P.tile([128, QR * W], BF16, tag="vb")
    nc.vector.memset(vb[64:128, :], 0.0)

    xsv = xsb.rearrange("p (r w) -> p r w", w=WP)
    w0v = w0sb.rearrange("p (t m) -> p t m", t=9)
    w1hv = w1hi.rearrange("p (t m) -> p t m", t=9)
    w1lv = w1lo.rearrange("p (t m) -> p t m", t=9)
    w2av = w2da.rearrange("p (t m) -> p t m", t=9)
    w2bv = w2db.rearrange("p (t m) -> p t m", t=9)

    # ---------------- conv0 ----------------
    with tc.tile_pool(name="ps_c0", bufs=4, space="PSUM") as pp0:
        for (r0, nr) in _row_blocks(YR):
            for mi, (m0, m1) in enumerate([(0, 128), (128, 192)]):
                ps = pp0.tile([128, 5, W], F32, tag="c0ps")
                mw = m1 - m0
                pb = 0 if mi == 0 else 64  # psum base partition (col tiling)
                for t in range(9):
                    dy, dx = t // 3 - 1, t % 3 - 1
                    nc.tensor.matmul(
                        ps[pb:pb + mw, 0:nr, :],
                        lhsT=w0v[:, t, m0:m1],
                        rhs=xsv[:, r0 + 1 + dy:r0 + 1 + dy + nr, 1 + dx:97 + dx],
                        start=(t == 0), stop=(t == 8))
                if mi == 0:
                    nc.scalar.activation(
                        out=xpool[0:64, r0:r0 + nr, 1:97], in_=ps[0:64, 0:nr, :],
                        func=mybir.ActivationFunctionType.Relu,
                        bias=sc0a[0:64, 1:2], scale=sc0a[0:64, 0:1])
                    if r0 < TR:
                        nc.scalar.activation(
                            out=ta[64:128, r0 + 1:r0 + 1 + nr, 1:97],
                            in_=ps[64:128, 0:nr, :],
                            func=mybir.ActivationFunctionType.Relu,
                            bias=sc0a[64:128, 1:2], scale=sc0a[64:128, 0:1])
                else:
                    nc.scalar.activation(
                        out=xpool[64:128, r0:r0 + nr, 1:97],
                        in_=ps[64:128, 0:nr, :],
                        func=mybir.ActivationFunctionType.Relu,
                        bias=sc0b[:, 1:2], scale=sc0b[:, 0:1])

    # ---------------- pools + bilinear ----------------
    plh = P.tile([128, YR, 48], BF16, tag="plh")
    pl = P.tile([128, 26, 48], BF16, tag="pl")
    vint = P.tile([128, TR, 48], BF16, tag="vint")
    nc.vector.tensor_tensor(out=plh[0:64], in0=xpool[0:64, :, 1:97:2],
                            in1=xpool[0:64, :, 2:98:2], op=mybir.AluOpType.max)
    nc.vector.tensor_tensor(out=plh[64:128], in0=xpool[64:128, :, 1:97:2],
                            in1=xpool[64:128, :, 2:98:2], op=mybir.AluOpType.add)
    nc.vector.tensor_tensor(out=pl[0:64], in0=plh[0:64, 0:52:2, :],
                            in1=plh[0:64, 1:52:2, :], op=mybir.AluOpType.max)
    nc.vector.tensor_tensor(out=pl[64:128], in0=plh[64:128, 0:52:2, :],
                            in1=plh[64:128, 1:52:2, :], op=mybir.AluOpType.add)

    # vertical bilinear (t rows 0..49)
    nc.vector.tensor_copy(out=vint[:, 0, :], in_=pl[:, 0, :])
    tmpv = P.tile([128, 25, 48], BF16, tag="tmpv")
    nc.vector.tensor_scalar(out=tmpv[:], in0=pl[:, 1:26, :], scalar1=0.25,
                            scalar2=None, op0=mybir.AluOpType.mult)
    nc.vector.scalar_tensor_tensor(
        out=vint[:, 1:50:2, :], in0=pl[:, 0:25, :], scalar=0.75,
        in1=tmpv[:], op0=mybir.AluOpType.mult, op1=mybir.AluOpType.add)
    nc.vector.tensor_scalar(out=tmpv[:, 0:24, :], in0=pl[:, 1:25, :], scalar1=0.75,
                            scalar2=None, op0=mybir.AluOpType.mult)
    nc.vector.scalar_tensor_tensor(
        out=vint[:, 2:49:2, :], in0=pl[:, 0:24, :], scalar=0.25,
        in1=tmpv[:, 0:24, :], op0=mybir.AluOpType.mult, op1=mybir.AluOpType.add)

    # horizontal bilinear into brs rows 1..50 cols 1..96 (br1 lower, br3 upper;
    # avgpool's 0.25 folded into the upper-partition constants)
    cA = P.tile([128, 2], F32, tag="cA")
    nc.vector.memset(cA[0:64, 0:1], 0.75)
    nc.vector.memset(cA[0:64, 1:2], 0.25)
    nc.vector.memset(cA[64:128, 0:1], 0.1875)
    nc.vector.memset(cA[64:128, 1:2], 0.0625)
    cC = P.tile([128, 1], F32, tag="cC")
    nc.vector.memset(cC[0:64, :], 1.0)
    nc.vector.memset(cC[64:128, :], 0.25)

    nc.vector.tensor_scalar(out=brs[:, 1:51, 1:2], in0=vint[:, :, 0:1],
                            scalar1=cC[:, 0:1], scalar2=None,
                            op0=mybir.AluOpType.mult)
    nc.vector.tensor_scalar(out=brs[:, 1:51, 96:97], in0=vint[:, :, 47:48],
                            scalar1=cC[:, 0:1], scalar2=None,
                            op0=mybir.AluOpType.mult)
    tmph = P.tile([128, TR, 47], BF16, tag="tmph")
    nc.vector.tensor_scalar(out=tmph[:], in0=vint[:, :, 1:48], scalar1=cA[:, 1:2],
                            scalar2=None, op0=mybir.AluOpType.mult)
    nc.vector.scalar_tensor_tensor(
        out=brs[:, 1:51, 2:96:2], in0=vint[:, :, 0:47], scalar=cA[:, 0:1],
        in1=tmph[:], op0=mybir.AluOpType.mult, op1=mybir.AluOpType.add)
    nc.vector.tensor_scalar(out=tmph[:], in0=vint[:, :, 1:48], scalar1=cA[:, 0:1],
                            scalar2=None, op0=mybir.AluOpType.mult)
    nc.vector.scalar_tensor_tensor(
        out=brs[:, 1:51, 3:96:2], in0=vint[:, :, 0:47], scalar=cA[:, 1:2],
        in1=tmph[:], op0=mybir.AluOpType.mult, op1=mybir.AluOpType.add)

    nc.gpsimd.dma_start(out=ta[0:64, 1:51, :], in_=brs[0:64, 1:51, :])
    nc.gpsimd.dma_start(out=tb[0:64, 1:51, :], in_=brs[64:128, 1:51, :])

    # ---------------- conv1 + attention prologue ----------------
    qk_blocks = _row_blocks(SR)
    v_blocks = _row_blocks(QR)
    grp_dst = [
        (0, 128, qa, sc1[0], qk_blocks),
        (128, 192, qb, sc1[1], qk_blocks),
        (192, 320, ka, sc1[2], qk_blocks),
        (320, 384, kb, sc1[3], qk_blocks),
        (384, 512, va, sc1[4], v_blocks),
        (512, 576, vb, sc1[5], v_blocks),
    ]

    with tc.tile_pool(name="ps_c1", bufs=2, space="PSUM") as pp1, \
         tc.tile_pool(name="ps_tr", bufs=2, space="PSUM") as ppt, \
         tc.tile_pool(name="ps_s", bufs=1, space="PSUM") as pps:

        def conv1_group(gi):
            m0, m1, dst, sc, blocks = grp_dst[gi]
            mw = m1 - m0
            dstv = dst.rearrange("p (r w) -> p r w", w=W)
            for (r0, nr) in blocks:
                ps = pp1.tile([128, 5, W], F32, tag="c1ps")
                first = True
                for t in range(9):
                    dy, dx = t // 3 - 1, t % 3 - 1
                    for (wv, rhs) in ((w1hv, ta), (w1lv, tb)):
                        nc.tensor.matmul(
                            ps[0:mw, 0:nr, :],
                            lhsT=wv[:, t, m0:m1],
                            rhs=rhs[:, r0 + 1 + dy:r0 + 1 + dy + nr,
                                    1 + dx:97 + dx],
                            start=first, stop=(t == 8 and rhs is tb))
                        first = False
                nc.scalar.activation(
                    out=dstv[0:mw, r0:r0 + nr, :], in_=ps[0:mw, 0:nr, :],
                    func=mybir.ActivationFunctionType.Relu,
                    bias=sc[:, 1:2], scale=sc[:, 0:1])

        for gi in range(4):
            conv1_group(gi)

        # sumsq(q,k) + AllReduce #1 (overlaps conv1 v groups)
        sq = P.tile([128, SR * W], BF16, tag="sq")
        qsqa = P.tile([128, 1], F32, tag="qsqa")
        qsqb = P.tile([64, 1], F32, tag="qsqb")
        ksqa = P.tile([128, 1], F32, tag="ksqa")
        ksqb = P.tile([64, 1], F32, tag="ksqb")
        for src, dst in ((qa, qsqa), (qb, qsqb), (ka, ksqa), (kb, ksqb)):
            p = src.shape[0]
            nc.vector.tensor_tensor(out=sq[0:p], in0=src[:], in1=src[:],
                                    op=mybir.AluOpType.mult)
            nc.vector.reduce_sum(out=dst[:], in_=sq[0:p],
                                 axis=mybir.AxisListType.X)
        nc.gpsimd.dma_start(out=cc1i[0, 0:128], in_=qsqa[:, 0])
        nc.gpsimd.dma_start(out=cc1i[0, 128:192], in_=qsqb[:, 0])
        nc.gpsimd.dma_start(out=cc1i[1, 0:128], in_=ksqa[:, 0])
        nc.gpsimd.dma_start(out=cc1i[1, 128:192], in_=ksqb[:, 0])
        nc.gpsimd.collective_compute(
            "AllReduce", mybir.AluOpType.add, replica_groups=GROUPS,
            ins=[cc1i[:]], outs=[cc1o[:]])

        for gi in range(4, 6):
            conv1_group(gi)

        # global norms -> scale k in place
        rsa = P.tile([128, 2], F32, tag="rsa")
        rsb = P.tile([64, 2], F32, tag="rsb")
        nc.gpsimd.dma_start(out=rsa[:], in_=bass.AP(
            tensor=cc1o, offset=0, ap=[[1, 128], [C, 2]]))
        nc.gpsimd.dma_start(out=rsb[:], in_=bass.AP(
            tensor=cc1o, offset=128, ap=[[1, 64], [C, 2]]))
        for rs in (rsa, rsb):
            nc.scalar.activation(out=rs[:], in_=rs[:],
                                 func=mybir.ActivationFunctionType.Sqrt)
            nc.vector.tensor_scalar(out=rs[:], in0=rs[:], scalar1=float(L2_EPS),
                                    scalar2=None, op0=mybir.AluOpType.max)
            nc.vector.reciprocal(out=rs[:], in_=rs[:])
        nc.vector.tensor_scalar(out=ka[:], in0=ka[:], scalar1=rsa[:, 1:2],
                                scalar2=None, op0=mybir.AluOpType.mult)
        nc.vector.tensor_scalar(out=kb[:], in0=kb[:], scalar1=rsb[:, 1:2],
                                scalar2=None, op0=mybir.AluOpType.mult)

        # transposes + S partial
        NCH = SR * W // 128
        spa = pps.tile([128, C], F32, tag="spa")
        spb = pps.tile([64, C], F32, tag="spb")
        qav = qa.rearrange("p (c k) -> p c k", k=128)
        qbv = qb.rearrange("p (c k) -> p c k", k=128)
        kav = ka.rearrange("p (c k) -> p c k", k=128)
        kbv = kb.rearrange("p (c k) -> p c k", k=128)
        for ci in range(NCH):
            tq = ppt.tile([128, C], BF16, tag="tq")
            tk = ppt.tile([128, C], BF16, tag="tk")
            nc.tensor.transpose(tq[:, 0:128], qav[:, ci, :], ident[:])
            nc.tensor.transpose(tq[:, 128:192], qbv[:, ci, :], ident[0:64, 0:64])
            nc.tensor.transpose(tk[:, 0:128], kav[:, ci, :], ident[:])
            nc.tensor.transpose(tk[:, 128:192], kbv[:, ci, :], ident[0:64, 0:64])
            qtc = ev.tile([128, C], BF16, tag="qtc")
            ktc = ev.tile([128, C], BF16, tag="ktc")
            nc.scalar.copy(out=qtc[:], in_=tq[:])
            nc.scalar.copy(out=ktc[:], in_=tk[:])
            nc.tensor.matmul(spa[:], lhsT=qtc[:, 0:128], rhs=ktc[:],
                             start=(ci == 0), stop=(ci == NCH - 1))
            nc.tensor.matmul(spb[:], lhsT=qtc[:, 128:192], rhs=ktc[:],
                             start=(ci == 0), stop=(ci == NCH - 1))
        ssa = P.tile([128, C], F32, tag="ssa")
        ssb = P.tile([64, C], F32, tag="ssb")
        nc.scalar.copy(out=ssa[:], in_=spa[:])
        nc.scalar.copy(out=ssb[:], in_=spb[:])
        nc.gpsimd.dma_start(out=cc2i[0:128, :], in_=ssa[:])
        nc.gpsimd.dma_start(out=cc2i[128:192, :], in_=ssb[:])
        nc.gpsimd.collective_compute(
            "AllReduce", mybir.AluOpType.add, replica_groups=GROUPS,
            ins=[cc2i[:]], outs=[cc2o[:]])

    # ---------------- softmax + P^T ----------------
    sfa = P.tile([128, C], F32, tag="sfa")
    sfb = P.tile([64, C], F32, tag="sfb")
    nc.gpsimd.dma_start(out=sfa[:], in_=cc2o[0:128, :])
    nc.gpsimd.dma_start(out=sfb[:], in_=cc2o[128:192, :])
    paf = P.tile([128, C], BF16, tag="paf")
    pbf = P.tile([64, C], BF16, tag="pbf")
    for sf, rs, pf in ((sfa, rsa, paf), (sfb, rsb, pbf)):
        p = sf.shape[0]
        rqt = ev.tile([128, 1], F32, tag="rqt")
        mx = ev.tile([128, 1], F32, tag="mx")
        sm = ev.tile([128, 1], F32, tag="sm")
        nc.vector.tensor_tensor(out=rqt[0:p], in0=rs[:, 0:1], in1=tmps[0:p],
                                op=mybir.AluOpType.mult)
        nc.vector.tensor_scalar(out=sf[:], in0=sf[:], scalar1=rqt[0:p],
                                scalar2=None, op0=mybir.AluOpType.mult)
        nc.vector.reduce_max(out=mx[0:p], in_=sf[:], axis=mybir.AxisListType.X)
        nc.vector.tensor_scalar(out=mx[0:p], in0=mx[0:p], scalar1=-1.0,
                                scalar2=None, op0=mybir.AluOpType.mult)
        nc.scalar.activation(out=sf[:], in_=sf[:],
                             func=mybir.ActivationFunctionType.Exp,
                             bias=mx[0:p], scale=1.0, accum_out=sm[0:p])
        nc.vector.reciprocal(out=sm[0:p], in_=sm[0:p])
        nc.vector.tensor_scalar(out=pf[:], in0=sf[:], scalar1=sm[0:p],
                                scalar2=None, op0=mybir.AluOpType.mult)

    pta = P.tile([128, C], BF16, tag="pta")
    ptb = P.tile([128, C], BF16, tag="ptb")
    nc.vector.memset(ptb[:], 0.0)
    with tc.tile_pool(name="ps_pt", bufs=2, space="PSUM") as ppm, \
         tc.tile_pool(name="ps_pv", bufs=2, space="PSUM") as ppv:
        tp1 = ppm.tile([128, C], BF16, tag="tp1")
        nc.tensor.transpose(tp1[:, 0:128], paf[:, 0:128], ident[:])
        nc.tensor.transpose(tp1[:, 128:192], pbf[:, 0:128], ident[0:64, 0:64])
        nc.scalar.copy(out=pta[:], in_=tp1[:])
        tp2 = ppm.tile([128, C], BF16, tag="tp1")
        nc.tensor.transpose(tp2[0:64, 0:128], paf[:, 128:192], ident[:])
        nc.tensor.transpose(tp2[0:64, 128:192], pbf[:, 128:192],
                            ident[0:64, 0:64])
        nc.scalar.copy(out=ptb[0:64, :], in_=tp2[0:64, :])

        # out = P @ v
        vav = va.rearrange("p (r w) -> p r w", w=W)
        vbv = vb.rearrange("p (r w) -> p r w", w=W)
        for (r0, nr) in v_blocks:
            po = ppv.tile([128, 5, W], F32, tag="po")
            po2 = ppv.tile([128, 5, W], F32, tag="po2")
            nc.tensor.matmul(po[:, 0:nr, :], lhsT=pta[:, 0:128],
                             rhs=vav[:, r0:r0 + nr, :], start=True, stop=False)
            nc.tensor.matmul(po[:, 0:nr, :], lhsT=ptb[:, 0:128],
                             rhs=vbv[:, r0:r0 + nr, :], start=False, stop=True)
            nc.tensor.matmul(po2[0:64, 0:nr, :], lhsT=pta[:, 128:192],
                             rhs=vav[:, r0:r0 + nr, :], start=True, stop=False)
            nc.tensor.matmul(po2[0:64, 0:nr, :], lhsT=ptb[:, 128:192],
                             rhs=vbv[:, r0:r0 + nr, :], start=False, stop=True)
            nc.scalar.copy(out=oa[:, r0 + 1:r0 + 1 + nr, 1:97],
                           in_=po[:, 0:nr, :])
            nc.scalar.copy(out=ob[0:64, r0 + 1:r0 + 1 + nr, 1:97],
                           in_=po2[0:64, 0:nr, :])

    # ---------------- depthwise conv + bias ----------------
    yv = yout.rearrange("c (r w) -> c r w", w=W)
    with tc.tile_pool(name="ps_dw", bufs=4, space="PSUM") as ppd:
        for (r0, nr) in _row_blocks(OR_):
            for (wv, src, b2t, mw, o0) in ((w2av, oa, b2a, 128, 0),
                                           (w2bv, ob, b2b, 64, 128)):
                ps = ppd.tile([128, 5, W], F32, tag="dwps")
                for t in range(9):
                    dy, dx = t // 3 - 1, t % 3 - 1
                    nc.tensor.matmul(
                        ps[0:mw, 0:nr, :],
                        lhsT=wv[:, t, 0:mw],
                        rhs=src[:, r0 + 1 + dy:r0 + 1 + dy + nr, 1 + dx:97 + dx],
                        start=(t == 0), stop=(t == 8))
                fo = ev.tile([128, 5, W], F32, tag="fo")
                nc.scalar.activation(out=fo[0:mw, 0:nr, :], in_=ps[0:mw, 0:nr, :],
                                     func=mybir.ActivationFunctionType.Identity,
                                     bias=b2t[:, 0:1], scale=1.0)
                nc.gpsimd.dma_start(out=yv[o0:o0 + mw, r0:r0 + nr, :],
                                    in_=fo[0:mw, 0:nr, :])
    ctx.close()


# ---------------- host side ----------------
_NC_CACHE = None
TRACE = False          # set by test.py for profiled runs
TRACE_KWARGS = {}
LAST_RESULTS = None


def _get_nc():
    global _NC_CACHE
    if _NC_CACHE is None:
        _NC_CACHE = build_nc()
    return _NC_CACHE


def _pack_weights(inp, flip):
    bf = ml_dtypes.bfloat16
    w0 = inp["w0"][:, :, ::-1, :] if flip else inp["w0"]
    w1 = inp["w1"][:, :, ::-1, :] if flip else inp["w1"]
    w2 = inp["w2"][:, :, ::-1, :] if flip else inp["w2"]

    w0t = np.zeros((64, 9, C), np.float32)
    for t in range(9):
        w0t[:, t] = w0[:, :, t // 3, t % 3].T
    s0 = inp["g0"] / np.sqrt(inp["v0"] + BN_EPS)
    t0 = inp["be0"] + (inp["b0"] - inp["m0"]) * s0
    sb0 = np.stack([s0, t0], axis=1).astype(np.float32)

    w1th = np.zeros((128, 9, C3), np.float32)
    w1tl = np.zeros((128, 9, C3), np.float32)
    for t in range(9):
        w1th[:, t] = w1[:, 0:128, t // 3, t % 3].T
        w1tl[0:64, t] = w1[:, 128:192, t // 3, t % 3].T
    s1 = inp["g1"] / np.sqrt(inp["v1"] + BN_EPS)
    t1 = inp["be1"] + (inp["b1"] - inp["m1"]) * s1
    sb1 = np.stack([s1, t1], axis=1).astype(np.float32)

    w2da = np.zeros((128, 9, 128), np.float32)
    w2db = np.zeros((128, 9, 128), np.float32)
    r64, r128 = np.arange(64), np.arange(128)
    for t in range(9):
        d = w2[:, 0, t // 3, t % 3]
        w2da[r128, t, r128] = d[0:128]
        w2db[r64, t, r64] = d[128:192]

    return {
        "w0t": np.ascontiguousarray(w0t.reshape(64, 9 * C)).astype(bf),
        "sb0": sb0,
        "w1th": np.ascontiguousarray(w1th.reshape(128, 9 * C3)).astype(bf),
        "w1tl": np.ascontiguousarray(w1tl.reshape(128, 9 * C3)).astype(bf),
        "sb1": sb1,
        "w2da": np.ascontiguousarray(w2da.reshape(128, 9 * 128)).astype(bf),
        "w2db": np.ascontiguousarray(w2db.reshape(128, 9 * 128)).astype(bf),
        "b2v": inp["b2"].reshape(C, 1).astype(np.float32),
    }


def kernel(**inputs):
    inputs = {k: np.asarray(v) for k, v in inputs.items()}
    x = inputs["x"]
    B = x.shape[0]
    bf = ml_dtypes.bfloat16
    packs = [_pack_weights(inputs, flip) for flip in (False, True)]
    tempv = np.asarray(inputs["temp"], np.float32).reshape(1, 1)

    in_maps = []
    for core in range(8):
        s, h = core // 2, core % 2
        xi = x[s]
        if h:
            xi = xi[:, ::-1, :]
        slab = np.zeros((64, XR, WP), np.float32)
        slab[:, 1:54, 1:97] = xi[:, 0:53, :]
        m = dict(packs[h])
        m["xs"] = np.ascontiguousarray(slab.reshape(64, XR * WP)).astype(bf)
        m["tempv"] = tempv
        in_maps.append(m)

    nc = _get_nc()
    res = run_bass_kernel_spmd(nc, in_maps, list(range(8)),
                               trace=TRACE, **TRACE_KWARGS)
    global LAST_RESULTS
    LAST_RESULTS = res
    out = np.zeros((B, C, 96, 96), np.float32)
    for core in range(8):
        s, h = core // 2, core % 2
        yc = res.results[core]["yout"].reshape(C, OR_, W)
        if h:
            out[s, :, 48:96] = yc[:, ::-1, :]
        else:
            out[s, :, 0:48] = yc
    return out

